# revision 6
# baseline (speedup 1.0000x reference)
"""Involution (B=4, C=256, H=W=56, K=7, G=16, reduction=4) on 8 trn2 NeuronCores.

Sharding: 8 shards = (batch b in 0..3) x (h-half in 0..1); each core computes
its [256, 28, 56] output slab.

v3 "(group, block)-major" design. Per core, partition p = g*7 + blk encodes
(group g in 0..15, 4-output-row block blk in 0..6). The per-pixel kernel w
stays COMPACT (no 16x channel broadcast): the DVE multiply reads it through a
stride-0 free-dim AP, replicating each (g,blk) row across the 16 channels of
the group for free.

Pipeline:
  1. stage1 (PE+Act):   t_ext = [relu(bn(W1 @ x)); ones]      [65, 1568] bf16
  2. w-gen (PE):        w_cmp[kt] = W2t[kt] @ t_ext  -> PSUM [112(g,kp), 1568]
     evac (Act):        PSUM -> SBUF bf16
  3. rearrange (DMA):   SBUF -> DRAM (shuffled) -> SBUF w_rT [112(g,blk),49,224]
  4. main loop (DVE+PE): per tap k: prod = x_win * w_rT[:,k] (bf16, 2x mode);
     PE identity-matmul accumulates prod into a 7-bank PSUM f32 accumulator.
     A few taps (D_TAPS) accumulate on DVE into a bf16 side-acc instead, to
     balance engine load.
  5. merge (DVE) + DMA out.
"""

import numpy as np
import ml_dtypes
from contextlib import ExitStack

import concourse.bass as bass
import concourse.bacc as bacc
import concourse.tile as tile
from concourse import mybir
from concourse.bass_utils import run_bass_kernel_spmd

BF16 = ml_dtypes.bfloat16

B, C, H, W = 4, 256, 56, 56
KK, G, PAD = 7, 16, 3
Cr = 64
EPS = 1e-5
HH = H // 2              # 28 rows per h-half shard
PH, PW = HH + 2 * PAD, W + 2 * PAD   # 34, 62 padded slab dims
NPIX = HH * W            # 1568 output pixels per shard
NCORES = 8

RB = 4                   # output rows per block
RIN = RB + 2 * PAD       # input rows held per partition slab (10)
NBLK = HH // RB          # 7 blocks
NP = G * NBLK            # 112 partitions used
PIXB = RB * W            # 224 pixels per block
FREE = 16 * PIXB         # 3584 elements per partition in the main loop
NTAP = KK * KK           # 49

# taps whose accumulation runs on DVE (into a bf16 side-acc) instead of PE.
# The first taps are chosen so DVE accumulates while w-gen still owns PSUM,
# hiding the pipeline prefix; they also rebalance PE vs DVE load.
D_TAPS = (0, 1, 2, 3, 4, 5, 6, 7, 8)

_CACHE = {}

TRACE = False
LAST_RESULT = None


def _build_nc():
    nc = bacc.Bacc("TRN2", target_bir_lowering=False, debug=False,
                   num_devices=NCORES)

    f32 = mybir.dt.float32
    bf16 = mybir.dt.bfloat16

    xc_d = nc.declare_dram_parameter("xc", [2, 128, PH, PW], bf16, isOutput=False)
    xr_d = nc.declare_dram_parameter("xr", [NP, G, RIN, PW], bf16, isOutput=False)
    w1t_d = nc.declare_dram_parameter("w1t", [2, 128, Cr], bf16, isOutput=False)
    b1p_d = nc.declare_dram_parameter("b1p", [Cr, 1], f32, isOutput=False)
    w2t_d = nc.declare_dram_parameter("w2t", [Cr + 1, KK, NP], bf16, isOutput=False)
    ident_d = nc.declare_dram_parameter("ident", [NP, NP], bf16, isOutput=False)
    out_d = nc.declare_dram_parameter("out", [NP, G, RB, W], f32, isOutput=True)

    wdram = nc.dram_tensor("wshuf", [NTAP, NP, PIXB], bf16)

    with tile.TileContext(nc) as tc, ExitStack() as ctx:
        const = ctx.enter_context(tc.tile_pool(name="const", bufs=1))
        xpool = ctx.enter_context(tc.tile_pool(name="x", bufs=1))
        tpool = ctx.enter_context(tc.tile_pool(name="t", bufs=1))
        wpool = ctx.enter_context(tc.tile_pool(name="w", bufs=1))

        # constants
        w1t_sb = const.tile([128, 2, Cr], bf16)
        for ch in range(2):
            nc.sync.dma_start(w1t_sb[:, ch, :], w1t_d[ch])
        b1p_sb = const.tile([Cr, 1], f32)
        nc.sync.dma_start(b1p_sb[:], b1p_d[:])
        w2t_sb = const.tile([Cr + 1, KK, NP], bf16)
        nc.sync.dma_start(w2t_sb[:], w2t_d[:])
        ident_sb = const.tile([NP, NP], bf16)
        nc.sync.dma_start(ident_sb[:], ident_d[:])

        # inputs
        xc_sb = []
        for ch in range(2):
            t_ = xpool.tile([128, PH, PW], bf16, tag=f"xc{ch}")
            nc.sync.dma_start(t_[:], xc_d[ch])
            xc_sb.append(t_)
        xr_sb = xpool.tile([NP, G, RIN, PW], bf16)
        nc.sync.dma_start(xr_sb[:], xr_d[:])

        # ---- stage 1: t_ext = [relu(W1p @ x + b1p); ones] in bf16 ----
        t_ext = tpool.tile([Cr + 1, NPIX], bf16)
        nc.vector.memset(t_ext[Cr:Cr + 1, :], 1.0)
        with tc.tile_pool(name="psum_t", bufs=2,
                          space=bass.MemorySpace.PSUM) as psum_t:
            NRW = 7          # 7 rows x 56 cols = 392 <= 512 (one bank)
            for q in range(HH // NRW):
                pt = psum_t.tile([Cr, NRW * W], f32)
                for ch in range(2):
                    rhs = xc_sb[ch][:, PAD + q * NRW:PAD + (q + 1) * NRW,
                                    PAD:PAD + W]
                    nc.tensor.matmul(pt[:], w1t_sb[:, ch, :], rhs,
                                     start=(ch == 0), stop=(ch == 1))
                nc.scalar.activation(
                    t_ext[0:Cr, q * NRW * W:(q + 1) * NRW * W],
                    pt[:], mybir.ActivationFunctionType.Relu,
                    bias=b1p_sb[:], scale=1.0)

        # ---- stage 2: compact w generation + DRAM-bounce rearrange ----
        # w_cmp[kt][g*7+kp, pix] = sum_o W2e[g*49+kt*7+kp, o] * t_ext[o, pix]
        w_cmp = [wpool.tile([NP, NPIX], bf16, name=f"wc{kt}") for kt in range(KK)]
        w_rT = wpool.tile([NP, NTAP, RB, W], bf16)
        WCH = 392            # 1568 px in 4 bank-aligned chunks of 392
        with tc.tile_pool(name="psum_w", bufs=2,
                          space=bass.MemorySpace.PSUM) as psum_w:
            for kt in range(KK):
                pw = psum_w.tile([NP, 4, 512], f32, tag="pw")
                for cch in range(4):
                    nc.tensor.matmul(pw[:, cch, 0:WCH], w2t_sb[:, kt, :],
                                     t_ext[:, cch * WCH:(cch + 1) * WCH],
                                     start=True, stop=True)
                nc.scalar.copy(
                    w_cmp[kt][:].rearrange("p (a x) -> p a x", a=4),
                    pw[:, :, 0:WCH])
                # shuffled store: dram row (g*7+blk) of tap k gets block blk
                for kp in range(KK):
                    k = kt * KK + kp
                    src = w_cmp[kt][kp:NP:KK].rearrange("g (b x) -> g b x",
                                                        b=NBLK)
                    dst = wdram[k].rearrange("(g b) x -> g b x", b=NBLK)
                    nc.scalar.dma_start(dst, src)
                # batched load of this septet into (g,blk)-partition layout
                dstl = w_rT[:, kt * KK:(kt + 1) * KK].rearrange(
                    "p k r c -> p k (r c)")
                srcl = wdram[kt * KK:(kt + 1) * KK].transpose([1, 0, 2])
                nc.sync.dma_start(dstl, srcl)

        # ---- stage 3: per-tap multiply + accumulate ----
        prodp = ctx.enter_context(tc.tile_pool(name="prod", bufs=4))
        saccp = ctx.enter_context(tc.tile_pool(name="sacc", bufs=1))
        outp = ctx.enter_context(tc.tile_pool(name="outp", bufs=1))

        pe_taps = [k for k in range(NTAP) if k not in D_TAPS]
        first_pe, last_pe = pe_taps[0], pe_taps[-1]
        sacc = saccp.tile([NP, G, RB, W], bf16, name="sacc") if D_TAPS else None
        first_d = D_TAPS[0] if D_TAPS else None

        with tc.tile_pool(name="psum_acc", bufs=1,
                          space=bass.MemorySpace.PSUM) as psum_acc:
            acc = psum_acc.tile([NP, FREE], f32)
            for k in range(NTAP):
                i, j = k // KK, k % KK
                xwin = xr_sb[:, :, i:i + RB, j:j + W]
                wtap = w_rT[:, k].unsqueeze(1).broadcast_to([NP, G, RB, W])
                if k in D_TAPS:
                    if k == first_d:
                        nc.vector.tensor_mul(sacc[:], xwin, wtap)
                    else:
                        dt_ = prodp.tile([NP, G, RB, W], bf16, tag="prod")
                        nc.vector.tensor_mul(dt_[:], xwin, wtap)
                        nc.vector.tensor_add(sacc[:], sacc[:], dt_[:])
                else:
                    pr = prodp.tile([NP, G, RB, W], bf16, tag="prod")
                    nc.vector.tensor_mul(pr[:], xwin, wtap)
                    prf = pr[:].rearrange("p a r c -> p (a r c)")
                    for cch in range(7):
                        nc.tensor.matmul(acc[:, cch * 512:(cch + 1) * 512],
                                         ident_sb[:],
                                         prf[:, cch * 512:(cch + 1) * 512],
                                         start=(k == first_pe),
                                         stop=(k == last_pe))

            of = outp.tile([NP, G, RB, W], f32)
            off = of[:].rearrange("p a r c -> p (a r c)")
            if D_TAPS:
                nc.vector.scalar_tensor_tensor(
                    off, acc[:], 1.0,
                    sacc[:].rearrange("p a r c -> p (a r c)"),
                    op0=mybir.AluOpType.mult, op1=mybir.AluOpType.add)
            else:
                nc.scalar.copy(off, acc[:])
            nc.sync.dma_start(out_d[:], of[:])

    nc.compile()
    return nc


def _prep_host_inputs(inputs, W1, b1, gamma, beta, mean, var, W2, b2):
    """Fold BN into W1/b1; build per-core rearranged inputs and W2 tiles."""
    scale = gamma / np.sqrt(var + EPS)
    shift = beta - mean * scale
    W1p = W1 * scale[:, None]
    b1p = (b1 * scale + shift).astype(np.float32).reshape(Cr, 1)
    w1t = np.ascontiguousarray(W1p.T.reshape(2, 128, Cr)).astype(BF16)

    # w2t[o, kt, g*7+kp] = W2e[g*49 + kt*7 + kp, o]
    W2e = np.concatenate([W2, b2[:, None]], axis=1)      # [784, 65]
    p_idx = np.arange(NP)
    kt_idx = np.arange(KK)
    rows = (p_idx[None, :] // KK) * NTAP + kt_idx[:, None] * KK \
        + (p_idx[None, :] % KK)                          # [7, 112]
    w2t = np.ascontiguousarray(W2e[rows].transpose(2, 0, 1)).astype(BF16)

    ident = np.eye(NP, dtype=np.float32).astype(BF16)

    xcs, xrs = [], []
    for core in range(NCORES):
        bt, hf = core // 2, core % 2
        slab = np.zeros((C, PH, PW), np.float32)
        r0 = hf * HH - PAD
        r1 = r0 + PH
        v0, v1 = max(r0, 0), min(r1, H)
        slab[:, v0 - r0:v1 - r0, PAD:PAD + W] = inputs[bt, :, v0:v1, :]
        xcs.append(slab.reshape(2, 128, PH, PW).astype(BF16))
        xg = slab.reshape(G, 16, PH, PW)
        xr = np.stack([xg[:, :, RB * blk:RB * blk + RIN]
                       for blk in range(NBLK)], axis=1)  # [16, 7, 16, 10, 62]
        xrs.append(np.ascontiguousarray(
            xr.reshape(NP, 16, RIN, PW)).astype(BF16))
    return xcs, xrs, w1t, b1p, w2t, ident


def kernel(inputs, W1, b1, gamma, beta, mean, var, W2, b2):
    global LAST_RESULT
    inputs = np.asarray(inputs, np.float32)
    if "nc" not in _CACHE:
        _CACHE["nc"] = _build_nc()
    nc = _CACHE["nc"]

    xcs, xrs, w1t, b1p, w2t, ident = _prep_host_inputs(
        inputs, np.asarray(W1, np.float32), np.asarray(b1, np.float32),
        np.asarray(gamma, np.float32), np.asarray(beta, np.float32),
        np.asarray(mean, np.float32), np.asarray(var, np.float32),
        np.asarray(W2, np.float32), np.asarray(b2, np.float32))

    in_maps = [{"xc": xcs[core], "xr": xrs[core], "w1t": w1t, "b1p": b1p,
                "w2t": w2t, "ident": ident} for core in range(NCORES)]
    res = run_bass_kernel_spmd(nc, in_maps, list(range(NCORES)), trace=TRACE)
    LAST_RESULT = res

    out = np.empty((B, C, H, W), np.float32)
    for core in range(NCORES):
        bt, hf = core // 2, core % 2
        r = res.results[core]["out"]                     # [112, 16, 4, 56]
        slab = r.reshape(G, NBLK, 16, RB, W).transpose(0, 2, 1, 3, 4)
        out[bt, :, hf * HH:(hf + 1) * HH, :] = slab.reshape(C, HH, W)
    return out


# revision 7
# speedup vs baseline: 1.0050x; 1.0050x over previous
"""Involution (B=4, C=256, H=W=56, K=7, G=16, reduction=4) on 8 trn2 NeuronCores.

Sharding: 8 shards = (batch b in 0..3) x (h-half in 0..1); each core computes
its [256, 28, 56] output slab.

v3 "(group, block)-major" design. Per core, partition p = g*7 + blk encodes
(group g in 0..15, 4-output-row block blk in 0..6). The per-pixel kernel w
stays COMPACT (no 16x channel broadcast): the DVE multiply reads it through a
stride-0 free-dim AP, replicating each (g,blk) row across the 16 channels of
the group for free.

Pipeline:
  1. stage1 (PE+Act):   t_ext = [relu(bn(W1 @ x)); ones]      [65, 1568] bf16
  2. w-gen (PE):        w_cmp[kt] = W2t[kt] @ t_ext  -> PSUM [112(g,kp), 1568]
     evac (Act):        PSUM -> SBUF bf16
  3. rearrange (DMA):   SBUF -> DRAM (shuffled) -> SBUF w_rT [112(g,blk),49,224]
  4. main loop (DVE+PE): per tap k: prod = x_win * w_rT[:,k] (bf16, 2x mode);
     PE identity-matmul accumulates prod into a 7-bank PSUM f32 accumulator.
     A few taps (D_TAPS) accumulate on DVE into a bf16 side-acc instead, to
     balance engine load.
  5. merge (DVE) + DMA out.
"""

import numpy as np
import ml_dtypes
from contextlib import ExitStack

import concourse.bass as bass
import concourse.bacc as bacc
import concourse.tile as tile
from concourse import mybir
from concourse.bass_utils import run_bass_kernel_spmd

BF16 = ml_dtypes.bfloat16

B, C, H, W = 4, 256, 56, 56
KK, G, PAD = 7, 16, 3
Cr = 64
EPS = 1e-5
HH = H // 2              # 28 rows per h-half shard
PH, PW = HH + 2 * PAD, W + 2 * PAD   # 34, 62 padded slab dims
NPIX = HH * W            # 1568 output pixels per shard
NCORES = 8

RB = 4                   # output rows per block
RIN = RB + 2 * PAD       # input rows held per partition slab (10)
NBLK = HH // RB          # 7 blocks
NP = G * NBLK            # 112 partitions used
PIXB = RB * W            # 224 pixels per block
FREE = 16 * PIXB         # 3584 elements per partition in the main loop
NTAP = KK * KK           # 49

# taps whose accumulation runs on DVE (into a bf16 side-acc) instead of PE.
# The first taps are chosen so DVE accumulates while w-gen still owns PSUM,
# hiding the pipeline prefix; they also rebalance PE vs DVE load.
D_TAPS = (0, 1, 2, 3, 4, 5, 6, 7, 8)

_CACHE = {}

TRACE = False
LAST_RESULT = None


def _build_nc():
    nc = bacc.Bacc("TRN2", target_bir_lowering=False, debug=False,
                   num_devices=NCORES)

    f32 = mybir.dt.float32
    bf16 = mybir.dt.bfloat16

    xc_d = nc.declare_dram_parameter("xc", [2, 128, PH, PW], bf16, isOutput=False)
    xr_d = nc.declare_dram_parameter("xr", [NP, G, RIN, PW], bf16, isOutput=False)
    w1t_d = nc.declare_dram_parameter("w1t", [2, 128, Cr], bf16, isOutput=False)
    b1p_d = nc.declare_dram_parameter("b1p", [Cr, 1], f32, isOutput=False)
    w2t_d = nc.declare_dram_parameter("w2t", [Cr + 1, KK, NP], bf16, isOutput=False)
    ident_d = nc.declare_dram_parameter("ident", [NP, NP], bf16, isOutput=False)
    out_d = nc.declare_dram_parameter("out", [NP, G, RB, W], f32, isOutput=True)

    wdram = nc.dram_tensor("wshuf", [NTAP, NP, PIXB], bf16)

    with tile.TileContext(nc) as tc, ExitStack() as ctx:
        const = ctx.enter_context(tc.tile_pool(name="const", bufs=1))
        xpool = ctx.enter_context(tc.tile_pool(name="x", bufs=1))
        tpool = ctx.enter_context(tc.tile_pool(name="t", bufs=1))
        wpool = ctx.enter_context(tc.tile_pool(name="w", bufs=1))

        # constants
        w1t_sb = const.tile([128, 2, Cr], bf16)
        for ch in range(2):
            nc.sync.dma_start(w1t_sb[:, ch, :], w1t_d[ch])
        b1p_sb = const.tile([Cr, 1], f32)
        nc.sync.dma_start(b1p_sb[:], b1p_d[:])
        w2t_sb = const.tile([Cr + 1, KK, NP], bf16)
        nc.sync.dma_start(w2t_sb[:], w2t_d[:])
        ident_sb = const.tile([NP, NP], bf16)
        nc.sync.dma_start(ident_sb[:], ident_d[:])

        # inputs
        xc_sb = []
        for ch in range(2):
            t_ = xpool.tile([128, PH, PW], bf16, tag=f"xc{ch}")
            nc.sync.dma_start(t_[:], xc_d[ch])
            xc_sb.append(t_)
        xr_sb = xpool.tile([NP, G, RIN, PW], bf16)
        nc.sync.dma_start(xr_sb[:], xr_d[:])

        # ---- stage 1: t_ext = [relu(W1p @ x + b1p); ones] in bf16 ----
        t_ext = tpool.tile([Cr + 1, NPIX], bf16)
        nc.vector.memset(t_ext[Cr:Cr + 1, :], 1.0)
        with tc.tile_pool(name="psum_t", bufs=2,
                          space=bass.MemorySpace.PSUM) as psum_t:
            NRW = 7          # 7 rows x 56 cols = 392 <= 512 (one bank)
            for q in range(HH // NRW):
                pt = psum_t.tile([Cr, NRW * W], f32)
                for ch in range(2):
                    rhs = xc_sb[ch][:, PAD + q * NRW:PAD + (q + 1) * NRW,
                                    PAD:PAD + W]
                    nc.tensor.matmul(pt[:], w1t_sb[:, ch, :], rhs,
                                     start=(ch == 0), stop=(ch == 1))
                nc.scalar.activation(
                    t_ext[0:Cr, q * NRW * W:(q + 1) * NRW * W],
                    pt[:], mybir.ActivationFunctionType.Relu,
                    bias=b1p_sb[:], scale=1.0)

        # ---- stage 2: compact w generation + DRAM-bounce rearrange ----
        # w_cmp[kt][g*7+kp, pix] = sum_o W2e[g*49+kt*7+kp, o] * t_ext[o, pix]
        w_cmp = [wpool.tile([NP, NPIX], bf16, name=f"wc{kt}") for kt in range(KK)]
        w_rT = wpool.tile([NP, NTAP, RB, W], bf16)
        WCH = 392            # 1568 px in 4 bank-aligned chunks of 392
        with tc.tile_pool(name="psum_w", bufs=2,
                          space=bass.MemorySpace.PSUM) as psum_w:
            for kt in range(KK):
                pw = psum_w.tile([NP, 4, 512], f32, tag="pw")
                for cch in range(4):
                    nc.tensor.matmul(pw[:, cch, 0:WCH], w2t_sb[:, kt, :],
                                     t_ext[:, cch * WCH:(cch + 1) * WCH],
                                     start=True, stop=True)
                nc.scalar.copy(
                    w_cmp[kt][:].rearrange("p (a x) -> p a x", a=4),
                    pw[:, :, 0:WCH])
                # shuffled store: dram row (g*7+blk) of tap k gets block blk
                for kp in range(KK):
                    k = kt * KK + kp
                    src = w_cmp[kt][kp:NP:KK].rearrange("g (b x) -> g b x",
                                                        b=NBLK)
                    dst = wdram[k].rearrange("(g b) x -> g b x", b=NBLK)
                    nc.sync.dma_start(dst, src)
                # batched load of this septet into (g,blk)-partition layout
                dstl = w_rT[:, kt * KK:(kt + 1) * KK].rearrange(
                    "p k r c -> p k (r c)")
                srcl = wdram[kt * KK:(kt + 1) * KK].transpose([1, 0, 2])
                nc.sync.dma_start(dstl, srcl)

        # ---- stage 3: per-tap multiply + accumulate ----
        prodp = ctx.enter_context(tc.tile_pool(name="prod", bufs=4))
        saccp = ctx.enter_context(tc.tile_pool(name="sacc", bufs=1))
        outp = ctx.enter_context(tc.tile_pool(name="outp", bufs=1))

        pe_taps = [k for k in range(NTAP) if k not in D_TAPS]
        first_pe, last_pe = pe_taps[0], pe_taps[-1]
        sacc = saccp.tile([NP, G, RB, W], bf16, name="sacc") if D_TAPS else None
        first_d = D_TAPS[0] if D_TAPS else None

        with tc.tile_pool(name="psum_acc", bufs=1,
                          space=bass.MemorySpace.PSUM) as psum_acc:
            acc = psum_acc.tile([NP, FREE], f32)
            for k in range(NTAP):
                i, j = k // KK, k % KK
                xwin = xr_sb[:, :, i:i + RB, j:j + W]
                wtap = w_rT[:, k].unsqueeze(1).broadcast_to([NP, G, RB, W])
                if k in D_TAPS:
                    if k == first_d:
                        nc.vector.tensor_mul(sacc[:], xwin, wtap)
                    else:
                        dt_ = prodp.tile([NP, G, RB, W], bf16, tag="prod")
                        nc.vector.tensor_mul(dt_[:], xwin, wtap)
                        nc.vector.tensor_add(sacc[:], sacc[:], dt_[:])
                else:
                    pr = prodp.tile([NP, G, RB, W], bf16, tag="prod")
                    nc.vector.tensor_mul(pr[:], xwin, wtap)
                    prf = pr[:].rearrange("p a r c -> p (a r c)")
                    for cch in range(7):
                        nc.tensor.matmul(acc[:, cch * 512:(cch + 1) * 512],
                                         ident_sb[:],
                                         prf[:, cch * 512:(cch + 1) * 512],
                                         start=(k == first_pe),
                                         stop=(k == last_pe))

            of = outp.tile([NP, G, RB, W], f32)
            off = of[:].rearrange("p a r c -> p (a r c)")
            if D_TAPS:
                nc.vector.scalar_tensor_tensor(
                    off, acc[:], 1.0,
                    sacc[:].rearrange("p a r c -> p (a r c)"),
                    op0=mybir.AluOpType.mult, op1=mybir.AluOpType.add)
            else:
                nc.scalar.copy(off, acc[:])
            nc.sync.dma_start(out_d[:], of[:])

    nc.compile()
    return nc


def _prep_host_inputs(inputs, W1, b1, gamma, beta, mean, var, W2, b2):
    """Fold BN into W1/b1; build per-core rearranged inputs and W2 tiles."""
    scale = gamma / np.sqrt(var + EPS)
    shift = beta - mean * scale
    W1p = W1 * scale[:, None]
    b1p = (b1 * scale + shift).astype(np.float32).reshape(Cr, 1)
    w1t = np.ascontiguousarray(W1p.T.reshape(2, 128, Cr)).astype(BF16)

    # w2t[o, kt, g*7+kp] = W2e[g*49 + kt*7 + kp, o]
    W2e = np.concatenate([W2, b2[:, None]], axis=1)      # [784, 65]
    p_idx = np.arange(NP)
    kt_idx = np.arange(KK)
    rows = (p_idx[None, :] // KK) * NTAP + kt_idx[:, None] * KK \
        + (p_idx[None, :] % KK)                          # [7, 112]
    w2t = np.ascontiguousarray(W2e[rows].transpose(2, 0, 1)).astype(BF16)

    ident = np.eye(NP, dtype=np.float32).astype(BF16)

    xcs, xrs = [], []
    for core in range(NCORES):
        bt, hf = core // 2, core % 2
        slab = np.zeros((C, PH, PW), np.float32)
        r0 = hf * HH - PAD
        r1 = r0 + PH
        v0, v1 = max(r0, 0), min(r1, H)
        slab[:, v0 - r0:v1 - r0, PAD:PAD + W] = inputs[bt, :, v0:v1, :]
        xcs.append(slab.reshape(2, 128, PH, PW).astype(BF16))
        xg = slab.reshape(G, 16, PH, PW)
        xr = np.stack([xg[:, :, RB * blk:RB * blk + RIN]
                       for blk in range(NBLK)], axis=1)  # [16, 7, 16, 10, 62]
        xrs.append(np.ascontiguousarray(
            xr.reshape(NP, 16, RIN, PW)).astype(BF16))
    return xcs, xrs, w1t, b1p, w2t, ident


def kernel(inputs, W1, b1, gamma, beta, mean, var, W2, b2):
    global LAST_RESULT
    inputs = np.asarray(inputs, np.float32)
    if "nc" not in _CACHE:
        _CACHE["nc"] = _build_nc()
    nc = _CACHE["nc"]

    xcs, xrs, w1t, b1p, w2t, ident = _prep_host_inputs(
        inputs, np.asarray(W1, np.float32), np.asarray(b1, np.float32),
        np.asarray(gamma, np.float32), np.asarray(beta, np.float32),
        np.asarray(mean, np.float32), np.asarray(var, np.float32),
        np.asarray(W2, np.float32), np.asarray(b2, np.float32))

    in_maps = [{"xc": xcs[core], "xr": xrs[core], "w1t": w1t, "b1p": b1p,
                "w2t": w2t, "ident": ident} for core in range(NCORES)]
    res = run_bass_kernel_spmd(nc, in_maps, list(range(NCORES)), trace=TRACE)
    LAST_RESULT = res

    out = np.empty((B, C, H, W), np.float32)
    for core in range(NCORES):
        bt, hf = core // 2, core % 2
        r = res.results[core]["out"]                     # [112, 16, 4, 56]
        slab = r.reshape(G, NBLK, 16, RB, W).transpose(0, 2, 1, 3, 4)
        out[bt, :, hf * HH:(hf + 1) * HH, :] = slab.reshape(C, HH, W)
    return out
